# revision 33
# baseline (speedup 1.0000x reference)
"""Trainium2 Bass kernel for nn_DiracGraphConv (GNN edge-softmax message passing).

Strategy (8 NeuronCores, SPMD, no collectives, no SWDGE):
  - Shard edges by DESTINATION node: core k owns local rows
    [k*12500, (k+1)*12500). Rows are degree-balanced (snake assignment
    after a degree sort) into NWIN=400 windows of <= 32 rows; 4 windows
    form a "quad" sharing a PSUM accumulator quarter-wise; quads are
    processed four at a time (a "group") so the big DVE instructions
    each cover 8192 tokens.
  - Host preprocessing materializes a per-token (edge) bf16 stream,
    partition-major: token slot (quad, chunkcol, p) holds
    [zhat_row | zhat_col | x_col | 1] (193 feats), zhat = z/||z||.
    Every window is padded to K*128 tokens (K = global max chunks);
    pad tokens carry rowloc=200 (matches no row).
  - Device per group: one linear DMA of the [128, 4*4K, 193] tile.
    Segment-indicator in [token, row, quad, chunkcol] layout so every
    DVE operand keeps a packed innermost dim (2x_1p eligible):
      st[p, r, j, cc] = (iota == rowloc)                      (DVE 2x)
      prod = zr * zc ; two tree-adds ; reduce -> num          (DVE)
      e = exp(alpha*num - |alpha|)   (per quad, overlapped)   (ACT)
      ste = st * e[bcast]                                     (DVE 2x)
    then 4K matmuls per quad, lhsT=ste slice (strided, M=32),
    rhs=[x|1] straight from the stream, accumulating into [64, 4, 65]
    PSUM tiles (4 window-halves per tile at partition base 0/32).
  - Finalize per quad (emitted one group late, software-pipelined):
    rec = 1/(denom+eps) batched per tile, y = [msgsum*rec | 1] via ACT
    copies with per-partition scale, bf16 PE transpose, matmul with
    [W^T; b], stream [128, 4*64] f32 out per group.  Host unshards via
    the row->slot permutation.
"""

import sys

sys.path.insert(0, "/opt/trn_rl_repo")

import numpy as np
import ml_dtypes

from concourse import bacc, bass, mybir, tile
from concourse.masks import make_identity

P = 128
F32 = mybir.dt.float32
BF16 = mybir.dt.bfloat16
BF_NP = ml_dtypes.bfloat16
EPS_DENOM = 1e-9
EPS_NORM = 1e-9

N_NODES = 100000
N_CORES = 8
NODES_PER_CORE = 12500
ROWS_PER_WIN = 32
NWIN = 400  # windows per core; NWIN*ROWS_PER_WIN = 12800 >= 12500
NQ = NWIN // 4  # quads (100; processed in groups of 4)
D = 64
FEAT = 3 * D + 1  # [zhat_row | zhat_col | x_col | 1]
PAD_ROWLOC = 200.0


def build_program(k_chunks: int, alpha: float):
    """One SPMD program for all cores. Inputs (per core):
    zall [P, NQ*CH*FEAT] bf16   token stream, partition-major (CH = 4K)
    rloc [P, NQ*CH]      bf16   row-in-window per token (200 = pad)
    iot  [P, 32*CH]      bf16   iot[p, r*CH+cc] = r
    wbs  [D+1, D]        bf16   [W^T; b]
    Output: out [P, NQ*D] f32 (quad-major columns)
    """
    K = k_chunks
    CH = 4 * K
    RW = ROWS_PER_WIN
    H = P // 2
    nc = bacc.Bacc("TRN2", target_bir_lowering=False, debug=False)

    zall = nc.dram_tensor("zall", [P, NQ * CH * FEAT], BF16, kind="ExternalInput").ap()
    rloc = nc.dram_tensor("rloc", [P, NQ * CH], BF16, kind="ExternalInput").ap()
    iot = nc.dram_tensor("iot", [P, RW * CH], BF16, kind="ExternalInput").ap()
    wbs_d = nc.dram_tensor("wbs", [D + 1, D], BF16, kind="ExternalInput").ap()
    out = nc.dram_tensor("out", [P, NQ * D], F32, kind="ExternalOutput").ap()

    zall_v = zall.rearrange("p (i c f) -> p i c f", i=NQ // 4, c=4 * CH)
    rloc_v = rloc.rearrange("p (g c) -> p g c", g=NQ)
    iot_v = iot.rearrange("p (r c) -> p r c", r=RW)
    out_v = out.rearrange("p (i d) -> p i d", i=NQ // 4)

    with tile.TileContext(nc) as tc:
        with (
            tc.tile_pool(name="const", bufs=1) as cpool,
            tc.tile_pool(name="load", bufs=3) as lpool,
            tc.tile_pool(name="work", bufs=2) as wpool,
            tc.tile_pool(name="smal", bufs=3) as spool,
            tc.tile_pool(name="fin", bufs=3) as fpool,
            tc.tile_pool(name="acc", bufs=2, space="PSUM") as ppool,
            tc.tile_pool(name="psmall", bufs=2, space="PSUM") as qpool,
        ):
            # ---- constants ----
            cb = cpool.tile([P, 1], F32, tag="cb")
            nc.vector.memset(cb[:], -abs(float(alpha)))
            identb = cpool.tile([P, P], BF16, tag="identb")
            make_identity(nc, identb[:])
            wbs = cpool.tile([D + 1, D], BF16, tag="wbs")
            nc.sync.dma_start(out=wbs[:], in_=wbs_d[:, :])
            rl = cpool.tile([P, 1, NQ, CH], BF16, tag="rl")
            nc.sync.dma_start(out=rl[:, 0, :, :], in_=rloc_v[:, :, :])
            ic = cpool.tile([P, RW, 1, CH], BF16, tag="ic")
            nc.sync.dma_start(out=ic[:, :, 0, :], in_=iot_v[:, :, :])

            def emit_finalize(sv):
                ost = sv["ost"]
                for t in range(2):
                    ps = sv["ps"][t]
                    rec4 = spool.tile([H, 4], F32, tag=f"rec4_{t}")
                    nc.vector.tensor_scalar_add(
                        rec4[:, :], ps[:, :, D : D + 1], EPS_DENOM
                    )
                    nc.vector.reciprocal(out=rec4[:], in_=rec4[:])
                    for jj in range(2):
                        j = 2 * t + jj
                        y = fpool.tile([P, D + 1], BF16, tag=f"y{j}")
                        nc.scalar.activation(
                            out=y[0:H, 0:D], in_=ps[:, 2 * jj, 0:D],
                            func=mybir.ActivationFunctionType.Copy,
                            scale=rec4[:, 2 * jj : 2 * jj + 1],
                        )
                        nc.scalar.activation(
                            out=y[H:P, 0:D], in_=ps[:, 2 * jj + 1, 0:D],
                            func=mybir.ActivationFunctionType.Copy,
                            scale=rec4[:, 2 * jj + 1 : 2 * jj + 2],
                        )
                        nc.vector.memset(y[:, D : D + 1], 1.0)
                        pt = qpool.tile([D + 1, P], BF16, tag="pt", space="PSUM")
                        nc.tensor.transpose(out=pt[:], in_=y[:], identity=identb[:])
                        lhs = fpool.tile([D + 1, P], BF16, tag=f"lhs{j}")
                        nc.scalar.copy(out=lhs[:], in_=pt[:])
                        yo = qpool.tile([P, D], F32, tag="yo", space="PSUM")
                        nc.tensor.matmul(
                            out=yo[:], lhsT=lhs[:], rhs=wbs[:], start=True,
                            stop=True,
                        )
                        nc.scalar.copy(out=ost[:, j, :], in_=yo[:])
                nc.sync.dma_start(
                    out=out_v[:, sv["i"], :],
                    in_=ost[:].rearrange("p a d -> p (a d)"),
                )

            pipe = []
            for i in range(NQ // 4):
                g0 = 4 * i
                zt = lpool.tile([P, 4 * CH, FEAT], BF16, tag="zt")
                nc.sync.dma_start(out=zt[:], in_=zall_v[:, i, :, :])

                prod = wpool.tile([P, 4 * CH, D], BF16, tag="prod")
                nc.vector.tensor_tensor(
                    out=prod[:], in0=zt[:, :, 0:D], in1=zt[:, :, D : 2 * D],
                    op=mybir.AluOpType.mult,
                )
                half = wpool.tile([P, 4 * CH, D // 2], BF16, tag="half")
                nc.vector.tensor_tensor(
                    out=half[:], in0=prod[:, :, 0 : D // 2],
                    in1=prod[:, :, D // 2 : D], op=mybir.AluOpType.add,
                )
                half2 = wpool.tile([P, 4 * CH, D // 4], BF16, tag="half2")
                nc.vector.tensor_tensor(
                    out=half2[:], in0=half[:, :, 0 : D // 4],
                    in1=half[:, :, D // 4 : D // 2], op=mybir.AluOpType.add,
                )
                num = spool.tile([P, 4, CH], F32, tag="num")
                e = spool.tile([P, 1, 4, CH], BF16, tag="e")
                h2v = half2[:].rearrange("p (a c) d -> p a c d", a=4)
                for j in range(4):
                    nc.vector.tensor_reduce(
                        out=num[:, j, :], in_=h2v[:, j, :, :],
                        axis=mybir.AxisListType.X, op=mybir.AluOpType.add,
                    )
                    nc.scalar.activation(
                        out=e[:, :, j, :], in_=num[:, j, :],
                        func=mybir.ActivationFunctionType.Exp,
                        bias=cb[:], scale=float(alpha),
                    )
                # eq has no data deps: fills DVE while ACT computes exp
                rlb = rl[:, :, g0 : g0 + 4, :].to_broadcast([P, RW, 4, CH])
                st = wpool.tile([P, RW, 4, CH], BF16, tag="st")
                nc.vector.tensor_tensor(
                    out=st[:], in0=rlb,
                    in1=ic[:].to_broadcast([P, RW, 4, CH]),
                    op=mybir.AluOpType.is_equal,
                )
                ste = wpool.tile([P, RW, 4, CH], BF16, tag="ste")
                for j in range(4):
                    nc.vector.tensor_tensor(
                        out=ste[:, :, j, :], in0=st[:, :, j, :],
                        in1=e[:, :, j, :].to_broadcast([P, RW, CH]),
                        op=mybir.AluOpType.mult,
                    )

                ost = fpool.tile([P, 4, D], F32, tag="ost")
                psE = ppool.tile([H, 4, D + 1], F32, tag="acc0", space="PSUM")
                psO = ppool.tile([H, 4, D + 1], F32, tag="acc1", space="PSUM")
                ps2 = [psE, psO]
                for j in range(4):
                    ps = ps2[j // 2]
                    for cc in range(CH):
                        q = cc // K
                        hi = 2 * (j % 2) + (0 if q < 2 else 1)
                        qh = q % 2
                        nc.tensor.matmul(
                            out=ps[qh * RW : (qh + 1) * RW, hi, :],
                            lhsT=ste[:, :, j, cc],
                            rhs=zt[:, j * CH + cc, 2 * D : 3 * D + 1],
                            start=(cc % K == 0), stop=(cc % K == K - 1),
                        )
                sv = {"i": i, "ps": ps2, "ost": ost}
                pipe.append(sv)
                if len(pipe) > 1:
                    emit_finalize(pipe.pop(0))
            for sv in pipe:
                emit_finalize(sv)

    nc.compile()
    return nc


def shard_inputs(x, z, edge_index, W, b):
    """Degree-balance rows into windows, materialize partition-major
    token streams, return (in_maps, K, slots) where slots give each
    node's output position for unsharding."""
    row = np.asarray(edge_index[0]).astype(np.int64)
    col = np.asarray(edge_index[1]).astype(np.int64)
    x = np.asarray(x, np.float32)
    z = np.asarray(z, np.float32)

    nrm = np.sqrt((z * z).sum(axis=1))
    zh = z / np.maximum(nrm, EPS_NORM)[:, None]
    zh_bf = zh.astype(BF_NP)
    x_bf = x.astype(BF_NP)

    deg = np.bincount(row, minlength=N_NODES)
    core_of = np.arange(N_NODES) // NODES_PER_CORE

    # --- per-core window assignment: snake over degree-sorted rows ---
    win = np.empty(N_NODES, np.int64)
    rowlocal = np.empty(N_NODES, np.int64)
    win_tokens = np.zeros((N_CORES, NWIN), np.int64)
    for k in range(N_CORES):
        nodes = np.arange(k * NODES_PER_CORE, (k + 1) * NODES_PER_CORE)
        order = np.argsort(-deg[nodes], kind="stable")
        sorted_nodes = nodes[order]
        i = np.arange(NODES_PER_CORE)
        pas, idx = i // NWIN, i % NWIN
        w = np.where(pas % 2 == 0, idx, NWIN - 1 - idx)
        win[sorted_nodes] = w
        rowlocal[sorted_nodes] = pas
        np.add.at(win_tokens[k], w, deg[sorted_nodes])
    assert rowlocal.max() < ROWS_PER_WIN
    K = max(1, -(-int(win_tokens.max()) // P))
    CH = 4 * K
    T = NQ * CH  # token columns per partition

    # --- place edges: rank within window -> (chunkcol, partition) ---
    E = row.shape[0]
    core_e = core_of[row]
    gw = core_e * NWIN + win[row]
    order_e = np.argsort(gw, kind="stable")
    cnt = np.bincount(gw, minlength=N_CORES * NWIN)
    starts = np.zeros(N_CORES * NWIN + 1, np.int64)
    np.cumsum(cnt, out=starts[1:])
    gw_o = gw[order_e]
    ranks = np.arange(E, dtype=np.int64) - starts[gw_o]
    ro, co = row[order_e], col[order_e]
    core_o = gw_o // NWIN
    w_o = gw_o % NWIN
    dest_part = ranks & 127
    dest_col = (w_o // 4) * CH + (w_o % 4) * K + (ranks >> 7)

    feat = np.empty((E, FEAT), BF_NP)
    feat[:, 0:D] = zh_bf[ro]
    feat[:, D : 2 * D] = zh_bf[co]
    feat[:, 2 * D : 3 * D] = x_bf[co]
    feat[:, 3 * D] = BF_NP(1.0)

    zall = np.zeros((N_CORES, P, T, FEAT), BF_NP)
    rloc = np.full((N_CORES, P, T), PAD_ROWLOC, BF_NP)
    zall[core_o, dest_part, dest_col] = feat
    rloc[core_o, dest_part, dest_col] = rowlocal[ro].astype(BF_NP)

    iot = np.ascontiguousarray(
        np.broadcast_to(
            np.repeat(np.arange(ROWS_PER_WIN, dtype=BF_NP), CH)[None, :],
            (P, ROWS_PER_WIN * CH),
        )
    )
    wbs = np.ascontiguousarray(
        np.concatenate(
            [np.asarray(W, np.float32).T, np.asarray(b, np.float32)[None, :]],
            axis=0,
        ).astype(BF_NP)
    )

    in_maps = [
        {
            "zall": np.ascontiguousarray(zall[k].reshape(P, T * FEAT)),
            "rloc": np.ascontiguousarray(rloc[k].reshape(P, T)),
            "iot": iot,
            "wbs": wbs,
        }
        for k in range(N_CORES)
    ]
    slot_part = (win % 4) * ROWS_PER_WIN + rowlocal
    slot_col = win // 4
    return in_maps, K, (slot_part, slot_col)


def unshard(results, slots):
    slot_part, slot_col = slots
    out_full = np.empty((N_NODES, D), np.float32)
    for k in range(N_CORES):
        o = np.asarray(results[k]["out"]).reshape(P, NQ, D)
        nodes = np.arange(k * NODES_PER_CORE, (k + 1) * NODES_PER_CORE)
        out_full[nodes] = o[slot_part[nodes], slot_col[nodes]]
    return out_full


def run(x, edge_index, z, W, b, alpha, bias_edge, trace=False):
    from concourse.bass_utils import run_bass_kernel_spmd

    in_maps, K, slots = shard_inputs(x, z, edge_index, W, b)
    nc = build_program(K, float(np.asarray(alpha)))
    res = run_bass_kernel_spmd(nc, in_maps, list(range(N_CORES)), trace=trace)
    return unshard(res.results, slots).astype(np.float32), res


def kernel(**inputs) -> np.ndarray:
    out, _ = run(
        inputs["x"],
        inputs["edge_index"],
        inputs["z"],
        inputs["W"],
        inputs["b"],
        inputs["alpha"],
        inputs["bias_edge"],
    )
    return out


# revision 34
# speedup vs baseline: 1.0431x; 1.0431x over previous
"""Trainium2 Bass kernel for nn_DiracGraphConv (GNN edge-softmax message passing).

Strategy (8 NeuronCores, SPMD, no collectives, no SWDGE):
  - Shard edges by DESTINATION node: core k owns local rows
    [k*12500, (k+1)*12500). Rows are degree-balanced (snake assignment
    after a degree sort) into NWIN=400 windows of <= 32 rows; 4 windows
    form a "quad" sharing a PSUM accumulator quarter-wise; quads are
    processed four at a time (a "group") so the big DVE instructions
    each cover 8192 tokens.
  - Host preprocessing materializes a per-token (edge) bf16 stream,
    partition-major: token slot (quad, chunkcol, p) holds
    [zhat_row | zhat_col | x_col | 1] (193 feats), zhat = z/||z||.
    Every window is padded to K*128 tokens (K = global max chunks);
    pad tokens carry rowloc=200 (matches no row).
  - Device per group: one linear DMA of the [128, 4*4K, 193] tile.
    Segment-indicator in [token, row, quad, chunkcol] layout so every
    DVE operand keeps a packed innermost dim (2x_1p eligible):
      st[p, r, j, cc] = (iota == rowloc)                      (DVE 2x)
      prod = zr * zc ; two tree-adds ; reduce -> num          (DVE)
      e = exp(alpha*num - |alpha|)   (per quad, overlapped)   (ACT)
      ste = st * e[bcast]                                     (DVE 2x)
    then 4K matmuls per quad, lhsT=ste slice (strided, M=32),
    rhs=[x|1] straight from the stream, accumulating into [64, 4, 65]
    PSUM tiles (4 window-halves per tile at partition base 0/32).
  - Finalize per quad (emitted one group late, software-pipelined):
    rec = 1/(denom+eps) batched per tile, y = [msgsum*rec | 1] via ACT
    copies with per-partition scale, bf16 PE transpose, matmul with
    [W^T; b], stream [128, 4*64] f32 out per group.  Host unshards via
    the row->slot permutation.
"""

import sys

sys.path.insert(0, "/opt/trn_rl_repo")

import numpy as np
import ml_dtypes

from concourse import bacc, bass, mybir, tile
from concourse.masks import make_identity

P = 128
F32 = mybir.dt.float32
BF16 = mybir.dt.bfloat16
BF_NP = ml_dtypes.bfloat16
EPS_DENOM = 1e-9
EPS_NORM = 1e-9

N_NODES = 100000
N_CORES = 8
NODES_PER_CORE = 12500
ROWS_PER_WIN = 32
NWIN = 400  # windows per core; NWIN*ROWS_PER_WIN = 12800 >= 12500
NQ = NWIN // 4  # quads (100; processed in groups of 4)
D = 64
FEAT = 3 * D + 1  # [zhat_row | zhat_col | x_col | 1]
PAD_ROWLOC = 200.0


def build_program(k_chunks: int, alpha: float):
    """One SPMD program for all cores. Inputs (per core):
    zall [P, NQ*CH*FEAT] bf16   token stream, partition-major (CH = 4K)
    rloc [P, NQ*CH]      bf16   row-in-window per token (200 = pad)
    iot  [P, 32*CH]      bf16   iot[p, r*CH+cc] = r
    wbs  [D+1, D]        bf16   [W^T; b]
    Output: out [P, NQ*D] f32 (quad-major columns)
    """
    K = k_chunks
    CH = 4 * K
    RW = ROWS_PER_WIN
    H = P // 2
    nc = bacc.Bacc("TRN2", target_bir_lowering=False, debug=False)

    zall = nc.dram_tensor("zall", [P, NQ * CH * FEAT], BF16, kind="ExternalInput").ap()
    rloc = nc.dram_tensor("rloc", [P, NQ * CH], BF16, kind="ExternalInput").ap()
    iot = nc.dram_tensor("iot", [P, RW * CH], BF16, kind="ExternalInput").ap()
    wbs_d = nc.dram_tensor("wbs", [D + 1, D], BF16, kind="ExternalInput").ap()
    out = nc.dram_tensor("out", [P, NQ * D], F32, kind="ExternalOutput").ap()

    zall_v = zall.rearrange("p (i c f) -> p i c f", i=NQ // 4, c=4 * CH)
    rloc_v = rloc.rearrange("p (g c) -> p g c", g=NQ)
    iot_v = iot.rearrange("p (r c) -> p r c", r=RW)
    out_v = out.rearrange("p (i d) -> p i d", i=NQ // 4)

    with tile.TileContext(nc) as tc:
        with (
            tc.tile_pool(name="const", bufs=1) as cpool,
            tc.tile_pool(name="load", bufs=3) as lpool,
            tc.tile_pool(name="work", bufs=2) as wpool,
            tc.tile_pool(name="smal", bufs=3) as spool,
            tc.tile_pool(name="fin", bufs=3) as fpool,
            tc.tile_pool(name="acc", bufs=3, space="PSUM") as ppool,
            tc.tile_pool(name="psmall", bufs=1, space="PSUM") as qpool,
        ):
            # ---- constants ----
            cb = cpool.tile([P, 1], F32, tag="cb")
            nc.vector.memset(cb[:], -abs(float(alpha)))
            identb = cpool.tile([P, P], BF16, tag="identb")
            make_identity(nc, identb[:])
            wbs = cpool.tile([D + 1, D], BF16, tag="wbs")
            nc.sync.dma_start(out=wbs[:], in_=wbs_d[:, :])
            rl = cpool.tile([P, 1, NQ, CH], BF16, tag="rl")
            nc.sync.dma_start(out=rl[:, 0, :, :], in_=rloc_v[:, :, :])
            ic = cpool.tile([P, RW, 1, CH], BF16, tag="ic")
            nc.sync.dma_start(out=ic[:, :, 0, :], in_=iot_v[:, :, :])

            def emit_finalize(sv):
                ost = sv["ost"]
                for t in range(2):
                    ps = sv["ps"][t]
                    rec4 = spool.tile([H, 4], F32, tag=f"rec4_{t}")
                    nc.vector.tensor_scalar_add(
                        rec4[:, :], ps[:, :, D : D + 1], EPS_DENOM
                    )
                    nc.vector.reciprocal(out=rec4[:], in_=rec4[:])
                    for jj in range(2):
                        j = 2 * t + jj
                        y = fpool.tile([P, D + 1], BF16, tag=f"y{j}")
                        nc.scalar.activation(
                            out=y[0:H, 0:D], in_=ps[:, 2 * jj, 0:D],
                            func=mybir.ActivationFunctionType.Copy,
                            scale=rec4[:, 2 * jj : 2 * jj + 1],
                        )
                        nc.scalar.activation(
                            out=y[H:P, 0:D], in_=ps[:, 2 * jj + 1, 0:D],
                            func=mybir.ActivationFunctionType.Copy,
                            scale=rec4[:, 2 * jj + 1 : 2 * jj + 2],
                        )
                        nc.vector.memset(y[:, D : D + 1], 1.0)
                        pt = qpool.tile([D + 1, P], BF16, tag="pt", space="PSUM")
                        nc.tensor.transpose(out=pt[:], in_=y[:], identity=identb[:])
                        lhs = fpool.tile([D + 1, P], BF16, tag=f"lhs{j}")
                        nc.scalar.copy(out=lhs[:], in_=pt[:])
                        yo = qpool.tile([P, D], F32, tag="yo", space="PSUM")
                        nc.tensor.matmul(
                            out=yo[:], lhsT=lhs[:], rhs=wbs[:], start=True,
                            stop=True,
                        )
                        nc.scalar.copy(out=ost[:, j, :], in_=yo[:])
                nc.sync.dma_start(
                    out=out_v[:, sv["i"], :],
                    in_=ost[:].rearrange("p a d -> p (a d)"),
                )

            pipe = []
            for i in range(NQ // 4):
                g0 = 4 * i
                zt = lpool.tile([P, 4 * CH, FEAT], BF16, tag="zt")
                nc.sync.dma_start(out=zt[:], in_=zall_v[:, i, :, :])

                prod = wpool.tile([P, 4 * CH, D], BF16, tag="prod")
                nc.vector.tensor_tensor(
                    out=prod[:], in0=zt[:, :, 0:D], in1=zt[:, :, D : 2 * D],
                    op=mybir.AluOpType.mult,
                )
                half = wpool.tile([P, 4 * CH, D // 2], BF16, tag="half")
                nc.vector.tensor_tensor(
                    out=half[:], in0=prod[:, :, 0 : D // 2],
                    in1=prod[:, :, D // 2 : D], op=mybir.AluOpType.add,
                )
                half2 = wpool.tile([P, 4 * CH, D // 4], BF16, tag="half2")
                nc.vector.tensor_tensor(
                    out=half2[:], in0=half[:, :, 0 : D // 4],
                    in1=half[:, :, D // 4 : D // 2], op=mybir.AluOpType.add,
                )
                num = spool.tile([P, 4, CH], F32, tag="num")
                e = spool.tile([P, 1, 4, CH], BF16, tag="e")
                h2v = half2[:].rearrange("p (a c) d -> p a c d", a=4)
                for j in range(4):
                    nc.vector.tensor_reduce(
                        out=num[:, j, :], in_=h2v[:, j, :, :],
                        axis=mybir.AxisListType.X, op=mybir.AluOpType.add,
                    )
                    nc.scalar.activation(
                        out=e[:, :, j, :], in_=num[:, j, :],
                        func=mybir.ActivationFunctionType.Exp,
                        bias=cb[:], scale=float(alpha),
                    )
                # eq has no data deps: fills DVE while ACT computes exp
                rlb = rl[:, :, g0 : g0 + 4, :].to_broadcast([P, RW, 4, CH])
                st = wpool.tile([P, RW, 4, CH], BF16, tag="st")
                nc.vector.tensor_tensor(
                    out=st[:], in0=rlb,
                    in1=ic[:].to_broadcast([P, RW, 4, CH]),
                    op=mybir.AluOpType.is_equal,
                )
                ste = wpool.tile([P, RW, 4, CH], BF16, tag="ste")
                for j in range(4):
                    nc.vector.tensor_tensor(
                        out=ste[:, :, j, :], in0=st[:, :, j, :],
                        in1=e[:, :, j, :].to_broadcast([P, RW, CH]),
                        op=mybir.AluOpType.mult,
                    )

                ost = fpool.tile([P, 4, D], F32, tag="ost")
                psE = ppool.tile([H, 4, D + 1], F32, tag="acc0", space="PSUM")
                psO = ppool.tile([H, 4, D + 1], F32, tag="acc1", space="PSUM")
                ps2 = [psE, psO]
                for j in range(4):
                    ps = ps2[j // 2]
                    for cc in range(CH):
                        q = cc // K
                        hi = 2 * (j % 2) + (0 if q < 2 else 1)
                        qh = q % 2
                        nc.tensor.matmul(
                            out=ps[qh * RW : (qh + 1) * RW, hi, :],
                            lhsT=ste[:, :, j, cc],
                            rhs=zt[:, j * CH + cc, 2 * D : 3 * D + 1],
                            start=(cc % K == 0), stop=(cc % K == K - 1),
                        )
                sv = {"i": i, "ps": ps2, "ost": ost}
                pipe.append(sv)
                if len(pipe) > 1:
                    emit_finalize(pipe.pop(0))
            for sv in pipe:
                emit_finalize(sv)

    nc.compile()
    return nc


def shard_inputs(x, z, edge_index, W, b):
    """Degree-balance rows into windows, materialize partition-major
    token streams, return (in_maps, K, slots) where slots give each
    node's output position for unsharding."""
    row = np.asarray(edge_index[0]).astype(np.int64)
    col = np.asarray(edge_index[1]).astype(np.int64)
    x = np.asarray(x, np.float32)
    z = np.asarray(z, np.float32)

    nrm = np.sqrt((z * z).sum(axis=1))
    zh = z / np.maximum(nrm, EPS_NORM)[:, None]
    zh_bf = zh.astype(BF_NP)
    x_bf = x.astype(BF_NP)

    deg = np.bincount(row, minlength=N_NODES)
    core_of = np.arange(N_NODES) // NODES_PER_CORE

    # --- per-core window assignment: snake over degree-sorted rows ---
    win = np.empty(N_NODES, np.int64)
    rowlocal = np.empty(N_NODES, np.int64)
    win_tokens = np.zeros((N_CORES, NWIN), np.int64)
    for k in range(N_CORES):
        nodes = np.arange(k * NODES_PER_CORE, (k + 1) * NODES_PER_CORE)
        order = np.argsort(-deg[nodes], kind="stable")
        sorted_nodes = nodes[order]
        i = np.arange(NODES_PER_CORE)
        pas, idx = i // NWIN, i % NWIN
        w = np.where(pas % 2 == 0, idx, NWIN - 1 - idx)
        win[sorted_nodes] = w
        rowlocal[sorted_nodes] = pas
        np.add.at(win_tokens[k], w, deg[sorted_nodes])
    assert rowlocal.max() < ROWS_PER_WIN
    K = max(1, -(-int(win_tokens.max()) // P))
    CH = 4 * K
    T = NQ * CH  # token columns per partition

    # --- place edges: rank within window -> (chunkcol, partition) ---
    E = row.shape[0]
    core_e = core_of[row]
    gw = core_e * NWIN + win[row]
    order_e = np.argsort(gw, kind="stable")
    cnt = np.bincount(gw, minlength=N_CORES * NWIN)
    starts = np.zeros(N_CORES * NWIN + 1, np.int64)
    np.cumsum(cnt, out=starts[1:])
    gw_o = gw[order_e]
    ranks = np.arange(E, dtype=np.int64) - starts[gw_o]
    ro, co = row[order_e], col[order_e]
    core_o = gw_o // NWIN
    w_o = gw_o % NWIN
    dest_part = ranks & 127
    dest_col = (w_o // 4) * CH + (w_o % 4) * K + (ranks >> 7)

    feat = np.empty((E, FEAT), BF_NP)
    feat[:, 0:D] = zh_bf[ro]
    feat[:, D : 2 * D] = zh_bf[co]
    feat[:, 2 * D : 3 * D] = x_bf[co]
    feat[:, 3 * D] = BF_NP(1.0)

    zall = np.zeros((N_CORES, P, T, FEAT), BF_NP)
    rloc = np.full((N_CORES, P, T), PAD_ROWLOC, BF_NP)
    zall[core_o, dest_part, dest_col] = feat
    rloc[core_o, dest_part, dest_col] = rowlocal[ro].astype(BF_NP)

    iot = np.ascontiguousarray(
        np.broadcast_to(
            np.repeat(np.arange(ROWS_PER_WIN, dtype=BF_NP), CH)[None, :],
            (P, ROWS_PER_WIN * CH),
        )
    )
    wbs = np.ascontiguousarray(
        np.concatenate(
            [np.asarray(W, np.float32).T, np.asarray(b, np.float32)[None, :]],
            axis=0,
        ).astype(BF_NP)
    )

    in_maps = [
        {
            "zall": np.ascontiguousarray(zall[k].reshape(P, T * FEAT)),
            "rloc": np.ascontiguousarray(rloc[k].reshape(P, T)),
            "iot": iot,
            "wbs": wbs,
        }
        for k in range(N_CORES)
    ]
    slot_part = (win % 4) * ROWS_PER_WIN + rowlocal
    slot_col = win // 4
    return in_maps, K, (slot_part, slot_col)


def unshard(results, slots):
    slot_part, slot_col = slots
    out_full = np.empty((N_NODES, D), np.float32)
    for k in range(N_CORES):
        o = np.asarray(results[k]["out"]).reshape(P, NQ, D)
        nodes = np.arange(k * NODES_PER_CORE, (k + 1) * NODES_PER_CORE)
        out_full[nodes] = o[slot_part[nodes], slot_col[nodes]]
    return out_full


def run(x, edge_index, z, W, b, alpha, bias_edge, trace=False):
    from concourse.bass_utils import run_bass_kernel_spmd

    in_maps, K, slots = shard_inputs(x, z, edge_index, W, b)
    nc = build_program(K, float(np.asarray(alpha)))
    res = run_bass_kernel_spmd(nc, in_maps, list(range(N_CORES)), trace=trace)
    return unshard(res.results, slots).astype(np.float32), res


def kernel(**inputs) -> np.ndarray:
    out, _ = run(
        inputs["x"],
        inputs["edge_index"],
        inputs["z"],
        inputs["W"],
        inputs["b"],
        inputs["alpha"],
        inputs["bias_edge"],
    )
    return out
